# revision 1
# baseline (speedup 1.0000x reference)
"""Trainium2 Bass kernel for nn_Grapher (GNN message passing block).

Strategy: pure data-parallel over batch B=64 -> 8 cores x 8 samples.
Per sample, the edge conv collapses algebraically:
  max_k relu(BN(W_ec @ [x_i; x_j - x_i]))
    = relu(A[:,n] + max_k B[:,idx[n,k]] + shift)
with A = (W1-W2)*se @ h, B = W2*se @ h, so only two 768x384x210 matmuls
plus a 9-neighbor gather-max instead of a 768x768x1890 matmul.
The KNN runs on a 210x210 cosine matrix via vector-engine max/max_index/
match_replace (top-8 + 9th).  Mean-over-K of the LoRA edge prompts
commutes with the 1x1 conv, and is computed with an adjacency one-hot
matmul.  All BN scales/shifts are folded into weights on the host.
"""

import sys
from contextlib import ExitStack

import numpy as np

sys.path.insert(0, "/opt/trn_rl_repo")

import ml_dtypes  # noqa: E402
import concourse.bass as bass  # noqa: E402
import concourse.bacc as bacc  # noqa: E402
import concourse.mybir as mybir  # noqa: E402
import concourse.tile as tile  # noqa: E402
from concourse.masks import make_identity  # noqa: E402

F32 = mybir.dt.float32
BF16 = mybir.dt.bfloat16
U32 = mybir.dt.uint32
AF = mybir.ActivationFunctionType
ALU = mybir.AluOpType

B, C, H, W = 64, 384, 14, 14
R, P, K = 32, 14, 9
H1, N = 15, 210
HW = H * W          # 196
EPS = 1e-5
NCORES = 8
SPC = B // NCORES   # samples per core = 8
NPAIRS = SPC // 2   # 4
CCH = C // 128      # 3 c-chunks
C2 = 2 * C          # 768
C2CH = C2 // 128    # 6
NT = (128, 82)      # node chunks: 210 = 128 + 82
NEG = -1.0e30
GELU_AF = AF.Gelu
DEBUG_DUMPS = False

_CACHE = {}


def _ceil(a, b):
    return (a + b - 1) // b


def _build_nc():
    nc = bacc.Bacc(
        "TRN2", target_bir_lowering=False, debug=False,
        enable_asserts=False, num_devices=NCORES,
    )
    d = {}
    di = {
        "x_d": ([NPAIRS, 128, CCH, 2, HW], F32),
        "wfc1t": ([128, CCH, C], F32),
        "bias1": ([128, CCH], F32),
        "prom": ([128, CCH, P], F32),
        "wdownt": ([128, CCH, R], F32),
        "bdown": ([R, 1], F32),
        "gp": ([R, C], F32),
        "wat": ([128, CCH, C2], BF16),
        "wbt": ([128, CCH, C2], BF16),
        "shifte": ([128, C2CH], F32),
        "wfc2t": ([128, C2CH, C], BF16),
        "wupt": ([R, C], F32),
        "shifto": ([128, CCH], F32),
    }
    for name, (shape, dt) in di.items():
        d[name] = nc.dram_tensor(name, shape, dt, kind="ExternalInput").ap()
    d["y_d"] = nc.dram_tensor(
        "y_d", [NPAIRS, 128, CCH, 2, HW], F32, kind="ExternalOutput"
    ).ap()
    if DEBUG_DUMPS:
        dbg = {
            "dbg_hp": ([128, CCH, 2, N], F32),
            "dbg_lrp": ([R, 2, N], F32),
            "dbg_hbp": ([128, CCH, 2, N], F32),
            "dbg_gs": ([128, 2, N], F32),
            "dbg_i9": ([128, 2, 9], U32),
            "dbg_ap": ([128, 2, C2], BF16),
            "dbg_bp": ([128, 2, C2], BF16),
            "dbg_gt": ([128, K, C2], BF16),
            "dbg_am": ([128, 2, C2], BF16),
            "dbg_rt": ([128, C2CH, 2, N], BF16),
            "dbg_lmp": ([R, 2, N], F32),
        }
        for name, (shape, dt) in dbg.items():
            d[name] = nc.dram_tensor(name, shape, dt, kind="ExternalOutput").ap()
    return nc, d


def _build_program():
    nc, d = _build_nc()
    with tile.TileContext(nc) as tc:
        with ExitStack() as ctx:
            _emit(ctx, tc, nc, d)
    nc.compile()
    return nc


def _emit(ctx, tc, nc, d):
    wp = ctx.enter_context(tc.tile_pool(name="weights", bufs=1))
    pp = ctx.enter_context(tc.tile_pool(name="pair", bufs=2))
    sp = ctx.enter_context(tc.tile_pool(name="samp", bufs=2))
    pmm = ctx.enter_context(tc.tile_pool(name="pmm", bufs=3, space="PSUM"))
    plm = ctx.enter_context(tc.tile_pool(name="plm", bufs=1, space="PSUM"))
    ptr = ctx.enter_context(tc.tile_pool(name="ptr", bufs=2, space="PSUM"))
    pab = ctx.enter_context(tc.tile_pool(name="pab", bufs=2, space="PSUM"))
    dp = ctx.enter_context(tc.tile_pool(name="dscratch", bufs=2, space="DRAM"))

    # ---- persistent weights ----
    def wload(name, shape, dt):
        t = wp.tile(shape, dt, name=name)
        nc.sync.dma_start(t[:], d[name])
        return t

    wfc1t = wload("wfc1t", [128, CCH, C], F32)
    bias1 = wload("bias1", [128, CCH], F32)
    prom = wload("prom", [128, CCH, P], F32)
    wdownt = wload("wdownt", [128, CCH, R], F32)
    bdown = wload("bdown", [R, 1], F32)
    gp = wload("gp", [R, C], F32)
    wat = wload("wat", [128, CCH, C2], BF16)
    wbt = wload("wbt", [128, CCH, C2], BF16)
    shifte = wload("shifte", [128, C2CH], F32)
    wfc2t = wload("wfc2t", [128, C2CH, C], BF16)
    wupt = wload("wupt", [R, C], F32)
    shifto = wload("shifto", [128, CCH], F32)

    identf = wp.tile([128, 128], F32, name="identf")
    make_identity(nc, identf[:, :])
    identb = wp.tile([128, 128], BF16, name="identb")
    nc.vector.tensor_copy(identb[:, :], identf[:, :])
    id08 = wp.tile([128, 128], F32, name="id08")
    nc.vector.tensor_scalar_mul(id08[:, :], identf[:, :], 0.8)
    ones = wp.tile([128, 1], F32, name="ones")
    nc.vector.memset(ones[:, :], 1.0)

    for pair in range(NPAIRS):
        _emit_pair(tc, nc, d, pair, locals())


def _emit_pair(tc, nc, d, pair, env):
    pp, sp, pmm, plm, ptr, pab, dp = (env[k] for k in ("pp", "sp", "pmm", "plm", "ptr", "pab", "dp"))
    wfc1t, bias1, prom, wdownt, bdown, gp = (
        env[k] for k in ("wfc1t", "bias1", "prom", "wdownt", "bdown", "gp"))
    wat, wbt, shifte, wfc2t, wupt, shifto = (
        env[k] for k in ("wat", "wbt", "shifte", "wfc2t", "wupt", "shifto"))
    identf, identb, id08, ones = (env[k] for k in ("identf", "identb", "id08", "ones"))

    # ---- load x pair ----
    xp = pp.tile([128, CCH, 2, HW], F32, tag="xp")
    nc.sync.dma_start(xp[:], d["x_d"][pair])

    # ---- fc1 (+BN fold) : h_raw [c, n] per sample ----
    hp = pp.tile([128, CCH, 2, N], F32, tag="hp")
    for jo in range(CCH):
        ps = pmm.tile([128, 2, HW], F32, tag="mm")
        for ji in range(CCH):
            nc.tensor.matmul(
                out=ps[:, :, :],
                lhsT=wfc1t[:, ji, jo * 128:(jo + 1) * 128],
                rhs=xp[:, ji, :, :],
                start=(ji == 0), stop=(ji == CCH - 1),
            )
        for s2 in range(2):
            nc.scalar.activation(
                hp[:, jo, s2, :HW], ps[:, s2, :], AF.Identity,
                bias=bias1[:, jo:jo + 1],
            )
    for s2 in range(2):
        nc.scalar.activation(hp[:, :, s2, HW:N], prom[:, :, :], AF.Copy)

    # ---- LoRA down + gelu : lr [r, n] ----
    lrp = pp.tile([R, 2, N], F32, tag="lrp")
    psl = pmm.tile([R, 2, N], F32, tag="mm")
    for ji in range(CCH):
        nc.tensor.matmul(
            out=psl[:, :, :], lhsT=wdownt[:, ji, :], rhs=hp[:, ji, :, :],
            start=(ji == 0), stop=(ji == CCH - 1),
        )
    nc.scalar.activation(lrp[:, :, :], psl[:, :, :], GELU_AF, bias=bdown[:, 0:1])

    # ---- blend: hb = 0.8*h + 0.2*gp^T @ lr  (both f32 and bf16 copies) ----
    hbp = pp.tile([128, CCH, 2, N], F32, tag="hbp")
    hbb = pp.tile([128, CCH, 2, N], BF16, tag="hbb")
    for jo in range(CCH):
        ps = pmm.tile([128, 2, N], F32, tag="mm")
        nc.tensor.matmul(out=ps[:, :, :], lhsT=gp[:, jo * 128:(jo + 1) * 128],
                         rhs=lrp[:, :, :], start=True, stop=False)
        nc.tensor.matmul(out=ps[:, :, :], lhsT=id08[:, :], rhs=hp[:, jo, :, :],
                         start=False, stop=True)
        nc.scalar.activation(hbp[:, jo, :, :], ps[:, :, :], AF.Copy)
        nc.vector.tensor_copy(hbb[:, jo, :, :], ps[:, :, :])

    # ---- column norms -> cinv ----
    hsq = pp.tile([128, CCH, 2, N], F32, tag="hsq")
    nc.scalar.activation(hsq[:, :, :, :], hbp[:, :, :, :], AF.Square)
    pss = pmm.tile([1, 2, N], F32, tag="mm")
    for ji in range(CCH):
        nc.tensor.matmul(out=pss[:, :, :], lhsT=ones[:, :], rhs=hsq[:, ji, :, :],
                         start=(ji == 0), stop=(ji == CCH - 1))

    if DEBUG_DUMPS and pair == 0:
        nc.sync.dma_start(d["dbg_hp"], hp[:])
        nc.sync.dma_start(d["dbg_lrp"], lrp[:])
        nc.sync.dma_start(d["dbg_hbp"], hbp[:])
    for s2 in range(2):
        _emit_sample(tc, nc, d, pair, s2, env, hp, lrp, hbp, hbb, pss)

    # ---- fc2 + ep (paired) ----
    reluT = env["_reluT"]
    lmp = env["_lmp"]
    for jo in range(CCH):
        ps = pmm.tile([128, 2, N], F32, tag="mm")
        for jc in range(C2CH):
            nc.tensor.matmul(
                out=ps[:, :, :], lhsT=wfc2t[:, jc, jo * 128:(jo + 1) * 128],
                rhs=reluT[:, jc, :, :], start=(jc == 0), stop=False,
            )
        nc.tensor.matmul(out=ps[:, :, :], lhsT=wupt[:, jo * 128:(jo + 1) * 128],
                         rhs=lmp[:, :, :], start=False, stop=True)
        tf = pp.tile([128, 2, HW], F32, tag="tf")
        nc.scalar.activation(tf[:, :, :], ps[:, :, :HW], AF.Identity,
                             bias=shifto[:, jo:jo + 1])
        yo = pp.tile([128, 2, HW], F32, tag="yo")
        nc.vector.tensor_add(yo[:, :, :], tf[:, :, :], xp[:, jo, :, :])
        nc.sync.dma_start(d["y_d"][pair, :, jo, :, :], yo[:, :, :])


def _emit_sample(tc, nc, d, pair, s2, env, hp, lrp, hbp, hbb, pss):
    pp, sp, pmm, plm, ptr, pab, dp = (env[k] for k in ("pp", "sp", "pmm", "plm", "ptr", "pab", "dp"))
    identf, identb = env["identf"], env["identb"]
    wat, wbt, shifte = env["wat"], env["wbt"], env["shifte"]

    # ---- cinv ----
    den = sp.tile([1, N], F32, tag="den")
    nc.scalar.activation(den[:, :], pss[:1, s2, :], AF.Sqrt)
    nc.vector.tensor_scalar_add(den[:, :], den[:, :], 1e-12)
    cinv = sp.tile([1, N], F32, tag="cinv")
    nc.vector.reciprocal(cinv[:, :], den[:, :])
    cbc = sp.tile([128, N], F32, tag="cbc")
    nc.gpsimd.partition_broadcast(cbc[:, :], cinv[:, :])

    # ---- xn = hb * cinv (column-normalized) ----
    xn = sp.tile([128, CCH, N], F32, tag="xn")
    for j in range(CCH):
        nc.vector.tensor_mul(xn[:, j, :], hbp[:, j, s2, :], cbc[:, :])

    # ---- G[n, m] = hb[:,n] . xn[:,m] ----
    gs = sp.tile([128, 2, N], F32, tag="gs")
    for i, ni in enumerate(NT):
        ps = pmm.tile([128, N], F32, tag="mm")
        for j in range(CCH):
            nc.tensor.matmul(
                out=ps[:ni, :],
                lhsT=hbp[:, j, s2, i * 128:i * 128 + ni],
                rhs=xn[:, j, :],
                start=(j == 0), stop=(j == CCH - 1),
            )
        nc.scalar.activation(gs[:ni, i, :], ps[:ni, :], AF.Copy)

    # ---- top-9 per row: top-8 (max/max_index) + 9th (match_replace) ----
    m8 = sp.tile([128, 2, 8], F32, tag="m8")
    i9 = sp.tile([128, 2, 9], U32, tag="i9")
    gm = sp.tile([128, 2, N], F32, tag="gm")
    m8b = sp.tile([128, 2, 8], F32, tag="m8b")
    i8b = sp.tile([128, 2, 8], U32, tag="i8b")
    adj = sp.tile([128, 2, N], F32, tag="adj")
    for i, ni in enumerate(NT):
        nc.vector.max(m8[:ni, i, :], gs[:ni, i, :])
        nc.vector.max_index(i9[:ni, i, 0:8], m8[:ni, i, :], gs[:ni, i, :])
        nc.vector.match_replace(gm[:ni, i, :], m8[:ni, i, :], gs[:ni, i, :], NEG)
        nc.vector.max(m8b[:ni, i, :], gm[:ni, i, :])
        nc.vector.max_index(i8b[:ni, i, :], m8b[:ni, i, :], gm[:ni, i, :])
        nc.vector.tensor_copy(i9[:ni, i, 8:9], i8b[:ni, i, 0:1])
        nc.vector.tensor_scalar(
            adj[:ni, i, :], gs[:ni, i, :], m8b[:ni, i, 0:1], None, op0=ALU.is_ge,
        )

    if DEBUG_DUMPS and pair == 0 and s2 == 0:
        nc.sync.dma_start(d["dbg_gs"], gs[:])
        nc.sync.dma_start(d["dbg_i9"], i9[:])
    # ---- A, B edge-conv halves (bf16), B -> DRAM for the gather ----
    Ap = sp.tile([128, 2, C2], BF16, tag="Ap")
    Bp = sp.tile([128, 2, C2], BF16, tag="Bp")
    bvd = dp.tile([N, C2], BF16, tag="bvd")
    for i, ni in enumerate(NT):
        for wt, dst in ((wat, Ap), (wbt, Bp)):
            for hf in range(2):
                ps = pab.tile([128, 384], F32, tag="ab")
                for j in range(CCH):
                    nc.tensor.matmul(
                        out=ps[:ni, :],
                        lhsT=hbb[:, j, s2, i * 128:i * 128 + ni],
                        rhs=wt[:, j, hf * 384:(hf + 1) * 384],
                        start=(j == 0), stop=(j == CCH - 1),
                    )
                nc.scalar.activation(
                    dst[:ni, i, hf * 384:(hf + 1) * 384], ps[:ni, :], AF.Copy)
        nc.sync.dma_start(bvd[i * 128:i * 128 + ni, :], Bp[:ni, i, :])

    # ---- gather 9 neighbor rows of B and max-merge ----
    am = sp.tile([128, 2, C2], BF16, tag="am")
    for i, ni in enumerate(NT):
        gt = sp.tile([128, K, C2], BF16, tag="gt")
        t1 = sp.tile([128, 4, C2], BF16, tag="t1")
        t2 = sp.tile([128, 2, C2], BF16, tag="t2")
        for k in range(K):
            nc.gpsimd.indirect_dma_start(
                out=gt[:ni, k, :], out_offset=None,
                in_=bvd[:, :],
                in_offset=bass.IndirectOffsetOnAxis(ap=i9[:ni, i, k:k + 1], axis=0),
            )
        if DEBUG_DUMPS and pair == 0 and s2 == 0 and i == 0:
            nc.sync.dma_start(d["dbg_gt"], gt[:])
        nc.vector.tensor_tensor(out=t1[:ni, :, :], in0=gt[:ni, 0:4, :],
                                in1=gt[:ni, 4:8, :], op=ALU.max)
        nc.vector.tensor_tensor(out=t2[:ni, :, :], in0=t1[:ni, 0:2, :],
                                in1=t1[:ni, 2:4, :], op=ALU.max)
        nc.vector.tensor_tensor(out=t1[:ni, 0, :], in0=t2[:ni, 0, :],
                                in1=t2[:ni, 1, :], op=ALU.max)
        nc.vector.tensor_tensor(out=t2[:ni, 0, :], in0=t1[:ni, 0, :],
                                in1=gt[:ni, 8, :], op=ALU.max)
        # am = A + max_k B
        nc.vector.tensor_add(am[:ni, i, :], Ap[:ni, i, :], t2[:ni, 0, :])

    if DEBUG_DUMPS and pair == 0 and s2 == 0:
        nc.sync.dma_start(d["dbg_ap"], Ap[:])
        nc.sync.dma_start(d["dbg_bp"], Bp[:])
        nc.sync.dma_start(d["dbg_am"], am[:])
    # ---- transpose am -> [c, n], relu(+shift_e) ----
    if s2 == 0:
        env["_reluT"] = pp.tile([128, C2CH, 2, N], BF16, tag="reluT", name="reluT")
    reluT = env["_reluT"]
    for cc in range(C2CH):
        for i, ni in enumerate(NT):
            pt = ptr.tile([128, 128], BF16, tag="tr")
            nc.tensor.transpose(
                pt[:, :ni], am[:ni, i, cc * 128:(cc + 1) * 128], identb[:ni, :ni])
            nc.scalar.activation(
                reluT[:, cc, s2, i * 128:i * 128 + ni], pt[:, :ni], AF.Relu,
                bias=shifte[:, cc:cc + 1],
            )

    # ---- lr^T and Adj^T transposes, lr_mean = (lr @ Adj^T)/9 ----
    lrT = sp.tile([128, 2, R], F32, tag="lrT")
    adjT = sp.tile([128, 2, N], F32, tag="adjT")
    for i, ni in enumerate(NT):
        pt = ptr.tile([128, 128], F32, tag="tr")
        nc.tensor.transpose(
            pt[:ni, :R], lrp[:, s2, i * 128:i * 128 + ni], identf[:R, :R])
        nc.scalar.activation(lrT[:ni, i, :], pt[:ni, :R], AF.Copy)
    for io, nio in enumerate(NT):
        for ii, nii in enumerate(NT):
            pt = ptr.tile([128, 128], F32, tag="tr")
            nc.tensor.transpose(
                pt[:nio, :nii],
                adj[:nii, ii, io * 128:io * 128 + nio],
                identf[:nii, :nii],
            )
            nc.scalar.activation(
                adjT[:nio, io, ii * 128:ii * 128 + nii], pt[:nio, :nii], AF.Copy)

    if s2 == 0:
        env["_lmp"] = pp.tile([R, 2, N], F32, tag="lmp", name="lmp")
        env["_pslm"] = plm.tile([R, 2, N], F32, tag="lm", name="pslm")
    lmp, pslm = env["_lmp"], env["_pslm"]
    for i, ni in enumerate(NT):
        nc.tensor.matmul(
            out=pslm[:, s2, :], lhsT=lrT[:ni, i, :], rhs=adjT[:ni, i, :],
            start=(i == 0), stop=(i == 1),
        )
    nc.scalar.activation(lmp[:, s2, :], pslm[:, s2, :], AF.Copy, scale=1.0 / 9.0)
    if DEBUG_DUMPS and pair == 0 and s2 == 1:
        nc.sync.dma_start(d["dbg_rt"], reluT[:])
        nc.sync.dma_start(d["dbg_lmp"], lmp[:])


# ======================= host side =======================

def _prep_inputs(inputs):
    f32 = np.float32
    bf = ml_dtypes.bfloat16
    s1 = (inputs["bn1_g"] / np.sqrt(inputs["bn1_v"] + EPS)).astype(f32)
    Wfc1 = (inputs["w_fc1"] * s1[:, None]).astype(f32)
    b1 = ((inputs["b_fc1"] - inputs["bn1_m"]) * s1 + inputs["bn1_b"]).astype(f32)
    se = (inputs["bne_g"] / np.sqrt(inputs["bne_v"] + EPS)).astype(f32)
    W1 = inputs["w_ec"][:, :C]
    W2 = inputs["w_ec"][:, C:]
    WA = ((W1 - W2) * se[:, None]).astype(f32)
    WB = (W2 * se[:, None]).astype(f32)
    shift_e = ((inputs["b_ec"] - inputs["bne_m"]) * se + inputs["bne_b"]).astype(f32)
    s2 = (inputs["bn2_g"] / np.sqrt(inputs["bn2_v"] + EPS)).astype(f32)
    Wfc2 = (0.8 * inputs["w_fc2"] * s2[:, None]).astype(f32)
    wup = (0.2 * inputs["w_up"]).astype(f32)
    shift_out = (0.8 * ((inputs["b_fc2"] - inputs["bn2_m"]) * s2 + inputs["bn2_b"])
                 + 0.2 * inputs["b_up"]).astype(f32)

    def chunk_pj(a, nch):  # [nch*128, ...] -> [128, nch, ...]
        return np.ascontiguousarray(
            a.reshape(nch, 128, *a.shape[1:]).transpose(1, 0, *range(2, a.ndim + 1)))

    w = {
        "wfc1t": chunk_pj(Wfc1.T.copy(), CCH),                  # [128,3,384]
        "bias1": chunk_pj(b1, CCH),                             # [128,3]
        "prom": chunk_pj(inputs["node_prompts"].astype(f32), CCH),
        "wdownt": chunk_pj(inputs["w_down"].T.astype(f32).copy(), CCH),
        "bdown": inputs["b_down"].astype(f32).reshape(R, 1),
        "gp": (0.2 * inputs["graph_prompt"]).astype(f32),       # [32,384]
        "wat": chunk_pj(WA.T.copy(), CCH).astype(bf),           # [128,3,768]
        "wbt": chunk_pj(WB.T.copy(), CCH).astype(bf),
        "shifte": chunk_pj(shift_e, C2CH),                      # [128,6]
        "wfc2t": chunk_pj(Wfc2.T.copy(), C2CH).astype(bf),      # [128,6,384]
        "wupt": wup.T.astype(f32).copy(),                       # [32,384]
        "shifto": chunk_pj(shift_out, CCH),                     # [128,3]
    }
    w = {k: np.ascontiguousarray(v) for k, v in w.items()}
    return w


def _shard_x(x):
    # -> per-core [NPAIRS, 128, CCH, 2, HW] f32
    shards = []
    for c in range(NCORES):
        xs = x[c * SPC:(c + 1) * SPC].reshape(SPC, C, HW)
        xs = xs.reshape(NPAIRS, 2, CCH, 128, HW).transpose(0, 3, 2, 1, 4)
        shards.append(np.ascontiguousarray(xs.astype(np.float32)))
    return shards


def _unshard_y(results):
    out = np.empty((B, C, H, W), np.float32)
    for c in range(NCORES):
        y = results[c]["y_d"]  # [NPAIRS,128,CCH,2,HW]
        ys = y.transpose(0, 3, 2, 1, 4).reshape(SPC, C, H, W)
        out[c * SPC:(c + 1) * SPC] = ys
    return out


def get_program():
    if "nc" not in _CACHE:
        _CACHE["nc"] = _build_program()
    return _CACHE["nc"]


def run(inputs, trace=False, **kw):
    from concourse.bass_utils import run_bass_kernel_spmd
    nc = get_program()
    w = _prep_inputs(inputs)
    shards = _shard_x(np.asarray(inputs["x"], np.float32))
    in_maps = [{**w, "x_d": shards[c]} for c in range(NCORES)]
    res = run_bass_kernel_spmd(nc, in_maps, list(range(NCORES)), trace=trace, **kw)
    return _unshard_y(res.results), res


def kernel(**inputs):
    y, _ = run(inputs)
    return y


if __name__ == "__main__":
    get_program()
    print("program built OK")

